# revision 4
# baseline (speedup 1.0000x reference)
"""Causal self-attention (ALiBi + QK-RMSNorm + subln) on 8 TRN2 NeuronCores.

Sharding: 8 cores = 2 batches x 4 head-groups (4 heads / 128 head-dim each).
Per core: QKV projection for its 512 features, attention for its 4 heads,
and a partial output projection (row slice of Wproj); host sums the 4
partials per batch.

Key structure:
- The softmax denominator is never computed: the post-attention subln
  (RMS norm over head_dim) cancels any per-query positive scale, so the
  attention output is only rescaled by a constant "pinfix" row (host
  precomputed) to keep squares in fp32 range.
- ALiBi locality: head slot 0 (steepest slopes, >=0.25) only attends ~2
  key tiles back; slot 1 ~6 tiles back; slots 2/3 attend fully.
- Head slot 0 pins the ALiBi exp factorization per 128-query block
  (PIN=64); slots 1-3 pin per 512-query chunk with a centered reference
  (PIN=-128), letting one big EXP per key tile.
- All rsqrt computed as exp(-0.5*ln(x)) so the scalar engine stays on a
  single activation table set (natural_log_exp) - no table reloads.
- Attention runs chunk-outer (ic) with subln + output projection
  interleaved per chunk, so projection matmuls fill the tensor engine
  while EXPs run.

All matmuls run as float32r (full-rate fp32 streaming with ~11-bit input
rounding, fp32 accumulation).
"""
import math

import numpy as np

import concourse.bacc as bacc
import concourse.bass as bass
import concourse.mybir as mybir
from concourse.tile import TileContext

F32 = mybir.dt.float32
F32R = mybir.dt.float32r
AF = mybir.ActivationFunctionType

B, T, C = 2, 2048, 2048
TC = 4          # 512-query chunks
H, D = 16, 128
HG = 4          # head groups = cores per batch
HPG = 4         # heads per group
F = HPG * D     # 512 per-core qkv features
EPS = 1e-5
NEG = -1.0e30
PIN0 = 64.0     # slot-0 per-128-block pin
PINJ = -128.0   # slots 1-3 per-512-chunk pin (centered)
MAXDIFF0 = 2    # slot 0 keeps query-block diffs 0..2


def _alibi_slopes(n_heads):
    def pow2(n):
        start = 2 ** (-(2 ** (-(math.log2(n) - 3))))
        return [start * start**i for i in range(n)]

    if math.log2(n_heads).is_integer():
        return pow2(n_heads)
    c = 2 ** math.floor(math.log2(n_heads))
    s = pow2(c)
    extra = _alibi_slopes(2 * c)
    return s + extra[0::2][: n_heads - c]


def _jt_lo(j, ic):
    if j == 0:
        return max(0, 4 * ic - 2)
    if j == 1:
        return max(0, 4 * ic - 6)
    return 0


def _build(debug=False):
    nc = bacc.Bacc("TRN2", target_bir_lowering=False)

    xt = nc.dram_tensor("xt", [C, T], F32R, kind="ExternalInput")
    wq = nc.dram_tensor("wq", [C, F], F32R, kind="ExternalInput")
    wk = nc.dram_tensor("wk", [C, F], F32R, kind="ExternalInput")
    wv = nc.dram_tensor("wv", [C, F], F32R, kind="ExternalInput")
    wp = nc.dram_tensor("wp", [F, C], F32R, kind="ExternalInput")
    bias_tab = nc.dram_tensor("bias_tab", [128, HPG * 16], F32, kind="ExternalInput")
    pinfix = nc.dram_tensor("pinfix", [128, HPG * 512], F32R, kind="ExternalInput")
    cmask = nc.dram_tensor("cmask", [128, 128], F32, kind="ExternalInput")
    wqk = nc.dram_tensor("wqk", [128, 1], F32, kind="ExternalInput")
    ones_c = nc.dram_tensor("ones_c", [128, 1], F32R, kind="ExternalInput")
    ones_r = nc.dram_tensor("ones_r", [1, 128], F32R, kind="ExternalInput")
    out = nc.dram_tensor("out", [T, C], F32, kind="ExternalOutput")

    kind_s = dict(kind="ExternalOutput") if debug else {}
    qt_s = nc.dram_tensor("qt_s", [F, T], F32R, **kind_s)
    kt_s = nc.dram_tensor("kt_s", [F, T], F32R, **kind_s)
    v_s = nc.dram_tensor("v_s", [T, F], F32R, **kind_s)

    with nc.allow_low_precision(reason="f32r rounding of matmul operands is intentional"), TileContext(nc) as tc:
        with (
            tc.tile_pool(name="consts", bufs=1) as consts,
            tc.tile_pool(name="psum", bufs=1, space="PSUM") as psum,
        ):
            bias_t = consts.tile([128, HPG * 16], F32, tag="bias_t")
            nc.sync.dma_start(out=bias_t, in_=bias_tab[:, :])
            mask_t = consts.tile([128, 128], F32, tag="mask_t")
            nc.sync.dma_start(out=mask_t, in_=cmask[:, :])
            wqk_t = consts.tile([128, 1], F32, tag="wqk_t")
            nc.sync.dma_start(out=wqk_t, in_=wqk[:, :])
            onesc_t = consts.tile([128, 1], F32R, tag="onesc_t")
            nc.sync.dma_start(out=onesc_t, in_=ones_c[:, :])
            onesr_t = consts.tile([1, 128], F32R, tag="onesr_t")
            nc.sync.dma_start(out=onesr_t, in_=ones_r[:, :])
            eps_c = consts.tile([128, 1], F32, tag="eps_c")
            nc.vector.memset(eps_c, EPS)
            eps128_r = consts.tile([1, 1], F32, tag="eps128_r")
            nc.vector.memset(eps128_r, 128.0 * EPS)
            ones_f = consts.tile([128, 2], F32, tag="ones_f")
            nc.vector.memset(ones_f, 1.0)
            ones_m = consts.tile([128, 2], F32R, tag="ones_m")
            nc.vector.tensor_copy(ones_m, ones_f)
            zeros_f = consts.tile([128, 384], F32, tag="zeros_f")
            nc.vector.memset(zeros_f, 0.0)
            zeros_r = consts.tile([128, 384], F32R, tag="zeros_r")
            nc.vector.tensor_copy(zeros_r, zeros_f)

            # rk columns for all (head, j-tile), filled during phase A
            rk_all = consts.tile([128, HPG * 16], F32, tag="rk_all")

            # ---------------- Phase A: QKV (+ q/k norm) -> DRAM scratch ---
            with (
                tc.tile_pool(name="wpool", bufs=1) as wpool,
                tc.tile_pool(name="xpool", bufs=1) as xpool,
                tc.tile_pool(name="stg", bufs=2) as stg,
            ):
                # weights split into 4 parts each so the first matmuls can
                # start as soon as ~1MB has landed (fast HAM warm-up)
                wq_p, wk_p, wv_p = [], [], []
                for kq in range(4):
                    t_ = wpool.tile([128, 4, F], F32R, tag=f"wq{kq}")
                    nc.sync.dma_start(
                        out=t_,
                        in_=wq.rearrange("(ct p) f -> p ct f", p=128)[
                            :, 4 * kq:4 * kq + 4, :],
                    )
                    wq_p.append(t_)
                for kq in range(4):
                    t_ = wpool.tile([128, 4, F], F32R, tag=f"wk{kq}")
                    nc.sync.dma_start(
                        out=t_,
                        in_=wk.rearrange("(ct p) f -> p ct f", p=128)[
                            :, 4 * kq:4 * kq + 4, :],
                    )
                    wk_p.append(t_)
                for kq in range(4):
                    t_ = wpool.tile([128, 4, F], F32R, tag=f"wv{kq}")
                    nc.sync.dma_start(
                        out=t_,
                        in_=wv.rearrange("(ct p) f -> p ct f", p=128)[
                            :, 4 * kq:4 * kq + 4, :],
                    )
                    wv_p.append(t_)

                xt_r = xt.rearrange("(ct p) t -> p ct t", p=128)
                for tch in range(TC):
                    x_p = []
                    for kq in range(4):
                        t_ = xpool.tile([128, 4, 512], F32R, tag=f"x{kq}",
                                        bufs=2)
                        nc.sync.dma_start(
                            out=t_,
                            in_=xt_r[:, 4 * kq:4 * kq + 4,
                                     tch * 512:(tch + 1) * 512],
                        )
                        x_p.append(t_)

                    # ---- q: project, rms-normalize (1/sqrt(D) folded), store
                    for ft in range(4):
                        ps = psum.tile([128, 512], F32, tag="big_ps", bufs=2)
                        for ct in range(16):
                            nc.tensor.matmul(
                                ps,
                                wq_p[ct // 4][:, ct % 4,
                                              ft * 128:(ft + 1) * 128],
                                x_p[ct // 4][:, ct % 4, :],
                                start=(ct == 0),
                                stop=(ct == 15),
                            )
                        st = stg.tile([128, 512], F32R, tag="st", bufs=3)
                        nc.scalar.copy(st, ps)
                        qsq = stg.tile([128, 512], F32R, tag="qsq")
                        nc.vector.tensor_mul(qsq, st.bitcast(F32), st.bitcast(F32))
                        ps_row = psum.tile([1, 512], F32, tag="row_ps", bufs=2)
                        nc.tensor.matmul(ps_row, onesc_t, qsq, start=True, stop=True)
                        # 1/sqrt via exp(-0.5*ln(x)) - keeps one ACT table set
                        lnq = stg.tile([1, 512], F32, tag="lnq")
                        nc.scalar.activation(
                            lnq, ps_row, AF.Ln, scale=1.0, bias=eps128_r
                        )
                        rq_row = stg.tile([1, 512], F32R, tag="rq_row")
                        nc.scalar.activation(rq_row, lnq, AF.Exp, scale=-0.5)
                        ps_b = psum.tile([128, 512], F32, tag="rowb_ps", bufs=2)
                        nc.tensor.matmul(ps_b, onesr_t, rq_row, start=True, stop=True)
                        qhat = stg.tile([128, 512], F32R, tag="qhat", bufs=3)
                        nc.vector.tensor_mul(qhat, st.bitcast(F32), ps_b)
                        nc.sync.dma_start(
                            out=qt_s[ft * 128:(ft + 1) * 128,
                                     tch * 512:(tch + 1) * 512],
                            in_=qhat,
                        )

                    # ---- k: project, fold wq*wk, rk columns, store
                    for ft in range(4):
                        ps = psum.tile([128, 512], F32, tag="big_ps", bufs=2)
                        for ct in range(16):
                            nc.tensor.matmul(
                                ps,
                                wk_p[ct // 4][:, ct % 4,
                                              ft * 128:(ft + 1) * 128],
                                x_p[ct // 4][:, ct % 4, :],
                                start=(ct == 0),
                                stop=(ct == 15),
                            )
                        st = stg.tile([128, 512], F32R, tag="st", bufs=3)
                        nc.scalar.copy(st, ps)
                        ksq = stg.tile([128, 512], F32R, tag="ksq")
                        nc.vector.tensor_mul(ksq, st.bitcast(F32), st.bitcast(F32))
                        for ts4 in range(4):
                            jt = tch * 4 + ts4
                            psk = psum.tile([128, 2], F32, tag="s_ps", bufs=2)
                            nc.tensor.matmul(
                                psk, ksq[:, ts4 * 128:(ts4 + 1) * 128], ones_m,
                                start=True, stop=True,
                            )
                            col = rk_all[:, ft * 16 + jt:ft * 16 + jt + 1]
                            lnk = stg.tile([128, 1], F32, tag="lnk")
                            nc.scalar.activation(
                                lnk, psk[:, 0:1], AF.Ln,
                                scale=1.0 / 128.0, bias=eps_c,
                            )
                            nc.scalar.activation(col, lnk, AF.Exp, scale=-0.5)
                        khat = stg.tile([128, 512], F32R, tag="khat_a", bufs=3)
                        nc.vector.tensor_scalar_mul(
                            khat, st.bitcast(F32), scalar1=wqk_t
                        )
                        nc.sync.dma_start(
                            out=kt_s[ft * 128:(ft + 1) * 128,
                                     tch * 512:(tch + 1) * 512],
                            in_=khat,
                        )

                    # ---- v: natural layout, store
                    for ts4 in range(4):
                        ps = psum.tile([128, 512], F32, tag="big_ps", bufs=2)
                        for ct in range(16):
                            nc.tensor.matmul(
                                ps,
                                x_p[ct // 4][:, ct % 4,
                                             ts4 * 128:(ts4 + 1) * 128],
                                wv_p[ct // 4][:, ct % 4, :],
                                start=(ct == 0),
                                stop=(ct == 15),
                            )
                        st = stg.tile([128, 512], F32R, tag="st", bufs=3)
                        nc.scalar.copy(st, ps)
                        nc.sync.dma_start(
                            out=v_s[(tch * 4 + ts4) * 128:(tch * 4 + ts4 + 1) * 128, :],
                            in_=st,
                        )

            # ------- Phase B: attention + subln + out-proj, chunk-outer ----
            with (
                tc.tile_pool(name="head", bufs=1) as head,
                tc.tile_pool(name="ppool", bufs=3) as ppool,
                tc.tile_pool(name="small", bufs=2) as small,
            ):
                v_sr = v_s.rearrange("(jt p) f -> p jt f", p=128)
                khat_by_h, v_by_h = {}, {}
                for h in range(HPG):
                    kh = head.tile([128, T], F32R, tag=f"khat{h}")
                    nc.sync.dma_start(out=kh, in_=kt_s[h * 128:(h + 1) * 128, :])
                    khat_by_h[h] = kh
                    vh = head.tile([128, 16, 128], F32R, tag=f"v_h{h}")
                    nc.sync.dma_start(out=vh, in_=v_sr[:, :, h * 128:(h + 1) * 128])
                    v_by_h[h] = vh
                pf_t = head.tile([128, HPG * 512], F32R, tag="pf_t")
                nc.sync.dma_start(out=pf_t, in_=pinfix[:, :])
                wp_t = head.tile([128, HPG, C], F32R, tag="wp_t")
                nc.sync.dma_start(
                    out=wp_t, in_=wp.rearrange("(ht p) c -> p ht c", p=128)
                )

                for ic in range(TC):
                    rstd_rows = {}
                    yf_ic = {}
                    for h in range(HPG):
                        qhat = small.tile([128, 512], F32R, tag="qhat", bufs=3)
                        nc.sync.dma_start(
                            out=qhat,
                            in_=qt_s[h * 128:(h + 1) * 128,
                                     ic * 512:(ic + 1) * 512],
                        )
                        rk = rk_all[:, h * 16:(h + 1) * 16]
                        jt_lo = _jt_lo(h, ic)
                        jt_hi = 4 * ic + 3
                        y_ps = psum.tile([128, 512], F32, tag="big_ps", bufs=2)
                        for jt in range(jt_lo, jt_hi + 1):
                            s_ps = psum.tile([128, 512], F32, tag="s_ps",
                                             bufs=2)
                            nc.tensor.matmul(
                                s_ps, khat_by_h[h][:, jt * 128:(jt + 1) * 128],
                                qhat, start=True, stop=True,
                            )
                            pt = ppool.tile([128, 512], F32R, tag="pt")
                            if h == 0:
                                # per-128-block pinning; only diffs 0..2 live
                                p_lo = max(0, jt - 4 * ic)
                                p_hi = min(3, jt - 4 * ic + MAXDIFF0)
                                if p_lo > 0:
                                    nc.vector.tensor_copy(
                                        pt[:, 0:p_lo * 128],
                                        zeros_r[:, 0:p_lo * 128],
                                    )
                                if p_hi < 3:
                                    w = (3 - p_hi) * 128
                                    nc.vector.tensor_copy(
                                        pt[:, (p_hi + 1) * 128:512],
                                        zeros_r[:, 0:w],
                                    )
                                for p in range(p_lo, p_hi + 1):
                                    diff = 4 * ic + p - jt
                                    src_b = s_ps[:, p * 128:(p + 1) * 128]
                                    if diff == 0:
                                        nc.vector.tensor_add(src_b, src_b, mask_t)
                                    nc.scalar.activation(
                                        pt[:, p * 128:(p + 1) * 128], src_b,
                                        AF.Exp,
                                        scale=rk[:, jt:jt + 1],
                                        bias=bias_t[:, 0 * 16 + diff:
                                                    0 * 16 + diff + 1],
                                    )
                            else:
                                kk = jt_hi - jt
                                i_lo = max(0, jt - 4 * ic)
                                if i_lo > 0:
                                    nc.vector.tensor_scalar_add(
                                        s_ps[:, 0:i_lo * 128],
                                        s_ps[:, 0:i_lo * 128], scalar1=NEG,
                                    )
                                if jt >= 4 * ic:
                                    p = jt - 4 * ic
                                    src_b = s_ps[:, p * 128:(p + 1) * 128]
                                    nc.vector.tensor_add(src_b, src_b, mask_t)
                                nc.scalar.activation(
                                    pt, s_ps, AF.Exp,
                                    scale=rk[:, jt:jt + 1],
                                    bias=bias_t[:, h * 16 + kk:
                                                h * 16 + kk + 1],
                                )
                            nc.tensor.matmul(
                                y_ps, v_by_h[h][:, jt, :], pt,
                                start=(jt == jt_lo), stop=(jt == jt_hi),
                                skip_group_check=True,
                            )
                        # finish: pinfix scale, squares, row-sum, rstd row
                        yf = head.tile([128, 512], F32R, tag=f"yfin{h}",
                                       bufs=2)
                        yf_ic[h] = yf
                        nc.vector.tensor_mul(
                            yf, y_ps, pf_t[:, h * 512:(h + 1) * 512].bitcast(F32)
                        )
                        ysq = small.tile([128, 512], F32R, tag="ysq")
                        nc.vector.tensor_mul(
                            ysq, yf.bitcast(F32), yf.bitcast(F32)
                        )
                        ysum_ps = psum.tile([1, 512], F32, tag="row_ps", bufs=2)
                        nc.tensor.matmul(ysum_ps, onesc_t, ysq, start=True, stop=True)
                        lny = small.tile([1, 512], F32, tag="lny")
                        nc.scalar.activation(
                            lny, ysum_ps, AF.Ln, scale=1.0 / 128.0, bias=eps_c[0:1, :]
                        )
                        rr = small.tile([1, 512], F32R, tag=f"rstd{h}", bufs=2)
                        nc.scalar.activation(rr, lny, AF.Exp, scale=-0.5)
                        rstd_rows[h] = rr

                    # subln scale + output projection for this chunk
                    for h in range(HPG):
                        bps = psum.tile([128, 512], F32, tag="rowb_ps", bufs=2)
                        nc.tensor.matmul(
                            bps, onesr_t, rstd_rows[h], start=True, stop=True
                        )
                        nc.vector.tensor_mul(
                            yf_ic[h], yf_ic[h].bitcast(F32), bps
                        )
                    for tt4 in range(4):
                        tt = 4 * ic + tt4
                        for cc in range(4):
                            ps = psum.tile([128, 512], F32, tag="big_ps", bufs=2)
                            for h in range(HPG):
                                nc.tensor.matmul(
                                    ps,
                                    yf_ic[h][:, tt4 * 128:(tt4 + 1) * 128],
                                    wp_t[:, h, cc * 512:(cc + 1) * 512],
                                    start=(h == 0),
                                    stop=(h == HPG - 1),
                                )
                            ot = ppool.tile([128, 512], F32, tag="ot")
                            nc.vector.tensor_copy(ot, ps)
                            nc.sync.dma_start(
                                out=out[tt * 128:(tt + 1) * 128,
                                        cc * 512:(cc + 1) * 512],
                                in_=ot,
                            )

    nc.compile()
    return nc


_NC_CACHE = None


def _get_nc():
    global _NC_CACHE
    if _NC_CACHE is None:
        _NC_CACHE = _build()
    return _NC_CACHE


def kernel_in_maps(x, Wq, Wk, Wv, Wproj, q_rms_w, k_rms_w, subln_w):
    slopes = _alibi_slopes(H)

    x = np.asarray(x, dtype=np.float32)
    Wq = np.asarray(Wq, dtype=np.float32)
    Wk = np.asarray(Wk, dtype=np.float32)
    Wv = np.asarray(Wv, dtype=np.float32)
    Wproj = np.asarray(Wproj, dtype=np.float32)
    q_rms_w = np.asarray(q_rms_w, dtype=np.float32)
    k_rms_w = np.asarray(k_rms_w, dtype=np.float32)
    subln_w = np.asarray(subln_w, dtype=np.float32)

    wqk = (q_rms_w * k_rms_w).reshape(128, 1)
    cmask = np.where(
        np.arange(128)[:, None] <= np.arange(128)[None, :], 0.0, NEG
    ).astype(np.float32)
    ones_c = np.ones((128, 1), np.float32)
    ones_r = np.ones((1, 128), np.float32)
    dj = np.arange(128, dtype=np.float64)
    i_sub = np.arange(512, dtype=np.float64)

    in_maps = []
    for b in range(B):
        xt = np.ascontiguousarray(x[b].T)
        for g in range(HG):
            heads = [g + 4 * j for j in range(HPG)]  # strided: slopes shrink with j
            csel = np.concatenate(
                [np.arange(hh * D, (hh + 1) * D) for hh in heads]
            )
            wproj_s = np.ascontiguousarray(
                Wproj[csel, :] * np.tile(subln_w, HPG)[:, None]
            )
            bias_tab = np.empty((128, HPG * 16), np.float32)
            pinfix = np.empty((128, HPG * 512), np.float32)
            for j, hh in enumerate(heads):
                slope = slopes[hh]
                pin = PIN0 if j == 0 else PINJ
                for kk in range(16):
                    bias_tab[:, j * 16 + kk] = slope * (dj - pin - 128.0 * kk)
                if j == 0:
                    pf = np.exp(-slope * ((i_sub % 128) - PIN0))
                else:
                    pf = np.exp(-slope * (i_sub - 384.0 - PINJ))
                pinfix[:, j * 512:(j + 1) * 512] = np.float32(pf)[None, :]
            in_maps.append({
                "xt": xt,
                "wq": np.ascontiguousarray(Wq[:, csel]),
                "wk": np.ascontiguousarray(Wk[:, csel]),
                "wv": np.ascontiguousarray(Wv[:, csel]),
                "wp": wproj_s,
                "bias_tab": bias_tab,
                "pinfix": pinfix,
                "cmask": cmask,
                "wqk": wqk,
                "ones_c": ones_c,
                "ones_r": ones_r,
            })

    return in_maps


def gather(results):
    outs = [r["out"] for r in results]
    final = np.stack(
        [sum(outs[b * HG + 1:(b + 1) * HG], outs[b * HG]) for b in range(B)]
    )
    return final.astype(np.float32)


def kernel(x, Wq, Wk, Wv, Wproj, q_rms_w, k_rms_w, subln_w):
    from concourse.bass_utils import run_bass_kernel_spmd

    in_maps = kernel_in_maps(x, Wq, Wk, Wv, Wproj, q_rms_w, k_rms_w, subln_w)
    res = run_bass_kernel_spmd(_get_nc(), in_maps, core_ids=list(range(8)))
    return gather(res.results)


if __name__ == "__main__":
    rng = np.random.default_rng(0)
    ins = {
        "x": rng.standard_normal((B, T, C), dtype=np.float32),
        "Wq": rng.standard_normal((C, H * D), dtype=np.float32) / math.sqrt(C),
        "Wk": rng.standard_normal((C, H * D), dtype=np.float32) / math.sqrt(C),
        "Wv": rng.standard_normal((C, H * D), dtype=np.float32) / math.sqrt(C),
        "Wproj": rng.standard_normal((H * D, C), dtype=np.float32) * 0.001,
        "q_rms_w": np.ones(D, np.float32),
        "k_rms_w": np.ones(D, np.float32),
        "subln_w": np.ones(D, np.float32),
    }
    y = kernel(**ins)
    print("kernel output", y.shape, y.dtype, float(np.abs(y).mean()))
